# revision 1
# baseline (speedup 1.0000x reference)
"""BehaviorAwareGCNLayer on 8 Trainium2 NeuronCores.

Math (reference):
    hx  = x @ W
    out[r] = (1/deg[r]) * sum_{e: row[e]=r} sim_w[e]*sigmoid(rep[row]+rep[col])*ns[col] * hx[col]
    out += sigmoid(rep) * (x @ W_self);  leaky_relu(out, 0.01)

Device strategy (destination sharding, no collectives):
  - By linearity, W is applied AFTER aggregation: agg[r] = sum coef_e * x[col_e],
    out[r] = (agg[r]/deg[r]) @ W + sigmoid(rep_r)*(x_r @ W_self).
  - Host does LAYOUT only (grouping/padding/fancy-index copies); all value
    math (sigmoid, products, sums, matmuls) happens on device.
  - Core c owns destination rows [c*12500, (c+1)*12500). Edges are grouped
    into runs by (core, 128-row destination block, 32768-row source
    col-range), padded to a 32-edge quantum with run capacities uniform
    across cores -> single SPMD program.
  - Blocks are striped into G groups; chunk order is (group, range)-major so
    each dma_gather instruction reads one 32768-row window of x with
    all-valid int16 indices (<=1024 per instruction, the HW ucode limit),
    while early block groups finish (and finalize) before the gather stream
    ends.
  - Per 128-edge chunk: coef on DVE/ACT, one-hot S[e, j] = (row_off[e] == j)
    in bf16, gathered x rows split into bf16 hi/lo pairs (fp32-grade
    accuracy), PE matmul per (chunk x run) segment accumulates in PSUM:
        psum[j, 0:65]    += sum_e S[e,j] * [coef_e * x_hi[col_e] | 1]
        psum[j, 65:130]  += sum_e S[e,j] * [coef_e * x_lo[col_e] | 0]
    Runs close into a [128, 98, 65] SBUF accumulator (deg in col 64).
  - Per block: normalize by deg, concat with sigmoid(rep)*x_block, one PE
    transpose + one matmul with [W; W_self] applies both weight matrices,
    leaky_relu, DMA out.
"""
import sys

if "/opt/trn_rl_repo" not in sys.path:
    sys.path.insert(0, "/opt/trn_rl_repo")

import numpy as np

P = 128
D = 64
N_NODES = 100000
N_CORES = 8
N_LOC = N_NODES // N_CORES            # 12500 destination rows per core
N_BLK = (N_LOC + P - 1) // P          # 98 blocks per core
LAST_VALID = N_LOC - (N_BLK - 1) * P  # 84 valid rows in last block
RANGE = 32768                         # int16-addressable source window
N_RANGES = (N_NODES + RANGE - 1) // RANGE  # 4
BATCH = 32                            # chunks per compute batch
GCH = 8                               # chunks per dma_gather (1024-idx HW limit)
QUANT = 32                            # run padding quantum (PE base_partition)
N_GRP = 4                             # block stripes (finalize overlap)
DUMMY_OFF = 1000.0                    # one-hot-killing row offset for pad slots


def _layout(cap32):
    """Derive the uniform slot layout from per-(block, range) capacities.

    cap32[b][r]: run capacity in edges (multiple of QUANT).
    """
    n_blk = len(cap32)
    n_ranges = len(cap32[0])
    grp_of = [min(b * N_GRP // n_blk, N_GRP - 1) for b in range(n_blk)]
    groups = [[b for b in range(n_blk) if grp_of[b] == g] for g in range(N_GRP)]

    run_start = [[0] * n_ranges for _ in range(n_blk)]
    run_par = [[0] * n_ranges for _ in range(n_blk)]
    spans = []   # (range, start_slot, end_slot), 128-aligned
    runs = []    # (start_slot, end_slot, block, parity)
    pos = 0
    for g in range(N_GRP):
        for r in range(n_ranges):
            span_start = pos
            k = 0
            for b in groups[g]:
                run_start[b][r] = pos
                run_par[b][r] = k & 1
                cap = int(cap32[b][r])
                if cap:
                    runs.append((pos, pos + cap, b, k & 1))
                    k += 1
                pos += cap
            pos = -(-pos // P) * P  # pad span to chunk boundary
            if pos > span_start:
                spans.append((r, span_start, pos))
    total_slots = pos
    n_chunks = total_slots // P

    # segments: (block, parity, is_start, is_stop); every matmul is full-K
    # base-0 with the one-hot window selecting the run's edges
    chunk_segs = [[] for _ in range(n_chunks)]
    blk_last_chunk = [0] * n_blk
    for (s, e, b, par) in runs:
        cs, ce = s // P, (e - 1) // P
        for ci in range(cs, ce + 1):
            chunk_segs[ci].append(
                (b, par, s >= ci * P, e <= (ci + 1) * P))
        blk_last_chunk[b] = max(blk_last_chunk[b], ce)
    return dict(total_slots=total_slots, run_start=run_start,
                run_par=run_par, spans=spans,
                chunk_segs=chunk_segs, blk_last_chunk=blk_last_chunk,
                n_chunks=n_chunks)


def _build_program(n_tab, n_blk, cap32, last_valid):
    """Emit + compile the single-core SPMD program."""
    import concourse.bacc as bacc
    import concourse.mybir as mybir
    import concourse.tile as tile
    from concourse.masks import make_identity

    f32 = mybir.dt.float32
    bf16 = mybir.dt.bfloat16
    i16 = mybir.dt.int16
    i32 = mybir.dt.int32

    lay = _layout(cap32)
    C = lay["n_chunks"]
    chunk_segs = lay["chunk_segs"]
    blk_last_chunk = lay["blk_last_chunk"]

    nc = bacc.Bacc("TRN2", target_bir_lowering=False, debug=False)

    x_d = nc.dram_tensor("x", [n_tab, D], f32, kind="ExternalInput")
    idx_d = nc.dram_tensor("idx16", [P, C * 8], i16, kind="ExternalInput")
    rowoff_d = nc.dram_tensor("rowoff_t", [P, C], bf16, kind="ExternalInput")
    sw_d = nc.dram_tensor("sw_t", [P, C], f32, kind="ExternalInput")
    reprow_d = nc.dram_tensor("reprow_t", [P, C], f32, kind="ExternalInput")
    repc_d = nc.dram_tensor("repc_t", [P, C], f32, kind="ExternalInput")
    nsc_d = nc.dram_tensor("nsc_t", [P, C], f32, kind="ExternalInput")
    repsh_d = nc.dram_tensor("rep_sh", [P, n_blk], f32, kind="ExternalInput")
    xself_d = nc.dram_tensor("x_self", [n_blk * P, D], f32, kind="ExternalInput")
    wcat_d = nc.dram_tensor("w_cat", [2 * D, D], f32, kind="ExternalInput")
    out_d = nc.dram_tensor("out", [n_blk * P, D], f32, kind="ExternalOutput")

    AL = mybir.AluOpType
    ACT = mybir.ActivationFunctionType

    with tile.TileContext(nc) as tc:
        with (
            tc.tile_pool(name="meta", bufs=1) as meta,
            tc.tile_pool(name="idxp", bufs=3) as idxp,
            tc.tile_pool(name="gather", bufs=3) as gpool,
            tc.tile_pool(name="work", bufs=3) as wpool,
            tc.tile_pool(name="onehot", bufs=3) as opool,
            tc.tile_pool(name="const", bufs=1) as cpool,
            tc.tile_pool(name="fin", bufs=3) as fpool,
            tc.tile_pool(name="psum", bufs=3, space="PSUM") as psum,
            tc.tile_pool(name="psumT", bufs=2, space="PSUM") as psumT,
        ):
            rowoff_s = meta.tile([P, C], bf16)
            sw_s = meta.tile([P, C], f32)
            reprow_s = meta.tile([P, C], f32)
            repc_s = meta.tile([P, C], f32)
            nsc_s = meta.tile([P, C], f32)
            repsh_s = meta.tile([P, n_blk], f32)
            acc_all = meta.tile([P, n_blk, D + 1], f32)
            wcat_s = cpool.tile([2 * D, D], f32)
            ident = cpool.tile([P, P], f32)
            iota_i = cpool.tile([P, 2 * P], i32)
            iota_f = cpool.tile([P, 2 * P], bf16)
            nc.sync.dma_start(out=rowoff_s[:], in_=rowoff_d[:])
            nc.sync.dma_start(out=sw_s[:], in_=sw_d[:])
            nc.sync.dma_start(out=reprow_s[:], in_=reprow_d[:])
            nc.sync.dma_start(out=repc_s[:], in_=repc_d[:])
            nc.sync.dma_start(out=nsc_s[:], in_=nsc_d[:])
            nc.sync.dma_start(out=repsh_s[:], in_=repsh_d[:])
            nc.sync.dma_start(out=wcat_s[:], in_=wcat_d[:])
            nc.vector.memset(acc_all[:].rearrange("p b d -> p (b d)"), 0.0)
            make_identity(nc, ident[:])
            nc.gpsimd.iota(iota_i[:], pattern=[[1, 2 * P]], base=0,
                           channel_multiplier=0)
            nc.vector.tensor_copy(out=iota_f[:], in_=iota_i[:])

            run_psum = {}  # block -> live psum tile for its current run

            def finalize_block(blk):
                valid = P if blk < n_blk - 1 else last_valid
                agg = acc_all[:, blk, :]
                recip = fpool.tile([P, 1], f32, tag="recip")
                nc.any.tensor_scalar_add(out=recip[:], in0=agg[:, D:D + 1],
                                         scalar1=1e-6)
                nc.vector.reciprocal(out=recip[:], in_=recip[:])
                xb = fpool.tile([P, D], f32, tag="xb")
                nc.sync.dma_start(out=xb[:], in_=xself_d[blk * P:(blk + 1) * P, :])
                srep = fpool.tile([P, 1], f32, tag="srep")
                nc.scalar.activation(srep[:], repsh_s[:, blk:blk + 1], ACT.Sigmoid)
                cat = fpool.tile([P, 2 * D], f32, tag="cat")
                nc.any.tensor_scalar_mul(out=cat[:, 0:D], in0=agg[:, 0:D],
                                         scalar1=recip[:])
                nc.any.tensor_scalar_mul(out=cat[:, D:2 * D], in0=xb[:],
                                         scalar1=srep[:])
                catT_ps = psumT.tile([P, P], f32, tag="catT")
                nc.tensor.transpose(out=catT_ps[:], in_=cat[:], identity=ident[:])
                catT = fpool.tile([P, P], f32, tag="catT_s")
                nc.vector.tensor_copy(out=catT[:], in_=catT_ps[:])
                out_ps = psumT.tile([P, D], f32, tag="out_ps")
                nc.tensor.matmul(out=out_ps[:], lhsT=catT[:], rhs=wcat_s[:],
                                 start=True, stop=True)
                outb = fpool.tile([P, D], f32, tag="outb")
                lk = fpool.tile([P, D], f32, tag="lk")
                nc.any.tensor_scalar_mul(out=lk[:], in0=out_ps[:], scalar1=0.01)
                nc.any.tensor_tensor(out=outb[:], in0=out_ps[:], in1=lk[:],
                                     op=AL.max)
                nc.sync.dma_start(out=out_d[blk * P:blk * P + valid, :],
                                  in_=outb[:valid, :])

            # batches: within gather spans, never crossing a range boundary
            batches = []  # (c0, nb, range)
            for (r, s0, s1) in lay["spans"]:
                cs, ce = s0 // P, s1 // P
                for c0 in range(cs, ce, BATCH):
                    batches.append((c0, min(BATCH, ce - c0), r))

            for (c0, nb, r) in batches:
                idx_t = idxp.tile([P, BATCH * 8], i16, tag="idx")
                nc.sync.dma_start(out=idx_t[:, :nb * 8],
                                  in_=idx_d[:, c0 * 8:(c0 + nb) * 8])
                xg = gpool.tile([P, BATCH, D], f32, tag="xg")
                for s in range(0, nb, GCH):
                    ns = min(GCH, nb - s)
                    nc.gpsimd.dma_gather(
                        out_ap=xg[:, s:s + ns, :], in_ap=x_d[r * RANGE:, :],
                        idxs_ap=idx_t[:, s * 8:(s + ns) * 8],
                        num_idxs=ns * P, num_idxs_reg=ns * P, elem_size=D)

                # coef = sw * sigmoid(rep_row + rep_col) * ns_col   [P, nb]
                coef = wpool.tile([P, BATCH], f32, tag="coef")
                nc.any.tensor_tensor(out=coef[:, :nb],
                                     in0=reprow_s[:, c0:c0 + nb],
                                     in1=repc_s[:, c0:c0 + nb], op=AL.add)
                nc.scalar.activation(coef[:, :nb], coef[:, :nb], ACT.Sigmoid)
                nc.any.tensor_tensor(out=coef[:, :nb], in0=coef[:, :nb],
                                     in1=sw_s[:, c0:c0 + nb], op=AL.mult)
                nc.any.tensor_tensor(out=coef[:, :nb], in0=coef[:, :nb],
                                     in1=nsc_s[:, c0:c0 + nb], op=AL.mult)
                nc.vector.tensor_tensor(
                    out=xg[:, :nb, :], in0=xg[:, :nb, :],
                    in1=coef[:, :nb].rearrange("p (b o) -> p b o", o=1)
                        .to_broadcast([P, nb, D]),
                    op=AL.mult)

                # bf16 hi/lo rhs: [hi(64) | 1 | lo(64) | 0]
                xs2 = wpool.tile([P, BATCH, 2 * (D + 1)], bf16, tag="xs2")
                nc.vector.tensor_copy(out=xs2[:, :nb, 0:D], in_=xg[:, :nb, :])
                nc.vector.memset(xs2[:, :nb, D:D + 1], 1.0)
                nc.vector.tensor_tensor(out=xs2[:, :nb, D + 1:2 * D + 1],
                                        in0=xg[:, :nb, :],
                                        in1=xs2[:, :nb, 0:D],
                                        op=AL.subtract)
                nc.vector.memset(xs2[:, :nb, 2 * D + 1:2 * D + 2], 0.0)

                oh = opool.tile([P, BATCH, 2 * P], bf16, tag="oh")
                nc.vector.tensor_tensor(
                    out=oh[:, :nb, :],
                    in0=rowoff_s[:, c0:c0 + nb]
                        .rearrange("p (b o) -> p b o", o=1)
                        .to_broadcast([P, nb, 2 * P]),
                    in1=iota_f[:].rearrange("p (b n) -> p b n", b=1)
                        .to_broadcast([P, nb, 2 * P]),
                    op=AL.is_equal)

                for i in range(nb):
                    ci = c0 + i
                    for (blk, par, is_start, is_stop) in chunk_segs[ci]:
                        if is_start:
                            run_psum[blk] = psum.tile(
                                [P, 2 * (D + 1)], f32, tag="agg", name="agg_ps")
                        nc.tensor.matmul(
                            out=run_psum[blk][:],
                            lhsT=oh[:, i, par * P:(par + 1) * P],
                            rhs=xs2[:, i, :],
                            start=is_start, stop=is_stop)
                        if is_stop:
                            nc.any.tensor_tensor(
                                out=acc_all[:, blk, :], in0=acc_all[:, blk, :],
                                in1=run_psum[blk][:, 0:D + 1], op=AL.add)
                            nc.any.tensor_tensor(
                                out=acc_all[:, blk, :], in0=acc_all[:, blk, :],
                                in1=run_psum[blk][:, D + 1:2 * (D + 1)],
                                op=AL.add)
                    for blk in range(n_blk):
                        if blk_last_chunk[blk] == ci:
                            finalize_block(blk)

    nc.compile()
    return nc


def _preprocess(x, edge_index, sim_weight, rep, node_signal):
    """Host-side layout: group edges into (core, dest block, col range) runs,
    pad to uniform 32-edge-quantum capacities, produce per-core arrays."""
    import ml_dtypes

    row = np.ascontiguousarray(edge_index[0]).astype(np.int64)
    col = np.ascontiguousarray(edge_index[1]).astype(np.int64)
    sw = np.ascontiguousarray(sim_weight).astype(np.float32)
    rep_f = np.ascontiguousarray(rep).astype(np.float32)
    ns_f = np.ascontiguousarray(node_signal).astype(np.float32)
    E = row.shape[0]

    core = row // N_LOC
    lrow = row - core * N_LOC
    blk = lrow >> 7
    off = (lrow & 127).astype(np.float32)
    rng_e = col // RANGE

    counts = np.zeros((N_CORES, N_BLK, N_RANGES), dtype=np.int64)
    np.add.at(counts, (core, blk, rng_e), 1)
    cap32 = (-(-counts.max(axis=0) // QUANT) * QUANT).astype(np.int64)

    lay = _layout(cap32)
    C = lay["n_chunks"]
    total = lay["total_slots"]
    run_start = np.array(lay["run_start"], dtype=np.int64)  # [N_BLK, N_RANGES]

    key = (core * N_BLK + blk) * N_RANGES + rng_e
    n_groups = N_CORES * N_BLK * N_RANGES
    order = np.argsort(key, kind="stable")
    gcounts = np.bincount(key, minlength=n_groups)
    group_start = np.zeros(n_groups + 1, dtype=np.int64)
    np.cumsum(gcounts, out=group_start[1:])
    rank = np.arange(E, dtype=np.int64) - group_start[key[order]]
    ko = key[order]
    core_o = ko // (N_BLK * N_RANGES)
    blk_o = (ko // N_RANGES) % N_BLK
    rng_o = ko % N_RANGES
    slot = core_o * total + run_start[blk_o, rng_o] + rank

    tot = N_CORES * total
    idx_flat = np.zeros(tot, dtype=np.int16)
    rowoff_p = np.full(tot, DUMMY_OFF, dtype=np.float32)
    sw_p = np.zeros(tot, dtype=np.float32)
    reprow_p = np.zeros(tot, dtype=np.float32)
    repc_p = np.zeros(tot, dtype=np.float32)
    nsc_p = np.zeros(tot, dtype=np.float32)
    idx_flat[slot] = (col[order] - rng_o * RANGE).astype(np.int16)
    run_par = np.array(lay["run_par"], dtype=np.int64)
    rowoff_p[slot] = off[order] + 128.0 * run_par[blk_o, rng_o]
    sw_p[slot] = sw[order]
    reprow_p[slot] = rep_f[row[order]]
    repc_p[slot] = rep_f[col[order]]
    nsc_p[slot] = ns_f[col[order]]

    def per_core(a):
        return np.ascontiguousarray(a.reshape(N_CORES, C, P).transpose(0, 2, 1))

    rowoff_t = per_core(rowoff_p).astype(ml_dtypes.bfloat16)
    sw_t = per_core(sw_p)
    reprow_t = per_core(reprow_p)
    repc_t = per_core(repc_p)
    nsc_t = per_core(nsc_p)

    idx_w = idx_flat.reshape(N_CORES, C * 8, 16).transpose(0, 2, 1)
    idx16 = np.ascontiguousarray(np.tile(idx_w, (1, 8, 1)))

    rep_pad = np.zeros((N_CORES, N_BLK * P), dtype=np.float32)
    for c in range(N_CORES):
        rep_pad[c, :N_LOC] = rep_f[c * N_LOC:(c + 1) * N_LOC]
    rep_sh = np.ascontiguousarray(
        rep_pad.reshape(N_CORES, N_BLK, P).transpose(0, 2, 1))

    x_f = np.ascontiguousarray(x).astype(np.float32)
    x_self = np.zeros((N_CORES, N_BLK * P, D), dtype=np.float32)
    for c in range(N_CORES):
        x_self[c, :N_LOC] = x_f[c * N_LOC:(c + 1) * N_LOC]

    return (cap32, x_f, idx16, rowoff_t, sw_t, reprow_t, repc_t, nsc_t,
            rep_sh, x_self)


_compiled = {}


def _get_program(cap32):
    key = (N_NODES, N_BLK, LAST_VALID, tuple(map(tuple, cap32.tolist())))
    if key not in _compiled:
        _compiled[key] = _build_program(N_NODES, N_BLK, cap32, LAST_VALID)
    return _compiled[key]


def run(x, edge_index, sim_weight, rep, node_signal, W, W_self, trace=False):
    from concourse.bass_utils import run_bass_kernel_spmd

    (cap32, x_f, idx16, rowoff_t, sw_t, reprow_t, repc_t, nsc_t, rep_sh,
     x_self) = _preprocess(x, edge_index, sim_weight, rep, node_signal)
    w_cat = np.ascontiguousarray(
        np.concatenate([np.asarray(W, dtype=np.float32),
                        np.asarray(W_self, dtype=np.float32)], axis=0))
    nc = _get_program(cap32)
    in_maps = []
    for c in range(N_CORES):
        in_maps.append({
            "x": x_f,
            "idx16": idx16[c],
            "rowoff_t": rowoff_t[c],
            "sw_t": sw_t[c],
            "reprow_t": reprow_t[c],
            "repc_t": repc_t[c],
            "nsc_t": nsc_t[c],
            "rep_sh": rep_sh[c],
            "x_self": x_self[c],
            "w_cat": w_cat,
        })
    res = run_bass_kernel_spmd(nc, in_maps, core_ids=list(range(N_CORES)),
                               trace=trace)
    out = np.concatenate(
        [res.results[c]["out"][:N_LOC] for c in range(N_CORES)], axis=0)
    return out, res


def kernel(x, edge_index, sim_weight, rep, node_signal, W, W_self):
    out, _ = run(x, edge_index, sim_weight, rep, node_signal, W, W_self)
    return out



# revision 3
# speedup vs baseline: 4.6873x; 4.6873x over previous
"""BehaviorAwareGCNLayer on 8 Trainium2 NeuronCores.

Math (reference):
    hx  = x @ W
    out[r] = (1/deg[r]) * sum_{e: row[e]=r} sim_w[e]*sigmoid(rep[row]+rep[col])*ns[col] * hx[col]
    out += sigmoid(rep) * (x @ W_self);  leaky_relu(out, 0.01)

Device strategy (destination sharding, no collectives):
  - By linearity, W is applied AFTER aggregation: agg[r] = sum coef_e * x[col_e],
    out[r] = (agg[r]/deg[r]) @ W + sigmoid(rep_r)*(x_r @ W_self).
  - Host does LAYOUT only (grouping/padding/fancy-index copies); all value
    math (sigmoid, products, sums, matmuls) happens on device.
  - Destination rows are grouped into 128-row blocks; blocks are dealt to the
    8 cores snake-wise by descending edge count, so per-slot capacities are
    nearly equal across cores -> single SPMD program, minimal padding.
  - Edges are grouped per (core, block-slot) into runs padded to a whole
    number of 128-edge chunks, so every chunk belongs to exactly one block.
  - The host pre-expands x[col_e] (bf16) into edge-slot order; the device
    STREAMS it with plain contiguous DMA (no gather ucode at all).
  - coef = sw*sigmoid(rep_row+rep_col)*ns_col for all slots is computed once
    up front from 4 streamed f32 metadata planes.
  - Per 128-edge chunk: one-hot S[e, j] = (row_off[e] == j) in bf16 against a
    materialized iota; rhs = [coef_e * x_col | 1] bf16; PE matmul accumulates
    psum[j, 0:65] over the block's run (deg lands in col 64).
  - Per block finalize: recip(deg), cat = [agg*recip | sigmoid(rep)*x_self]
    in bf16 (ACT engine), one PE transpose + one matmul with [W; W_self]
    applies both weights, leaky_relu, DMA out. Output rows are re-assembled
    on host per the block permutation.
"""
import sys

if "/opt/trn_rl_repo" not in sys.path:
    sys.path.insert(0, "/opt/trn_rl_repo")

import numpy as np

P = 128
D = 64
N_NODES = 100000
N_CORES = 8
N_BLK_G = (N_NODES + P - 1) // P      # 782 global 128-row blocks
N_SLOT = (N_BLK_G + N_CORES - 1) // N_CORES  # 98 block slots per core
N_BLK_T = N_SLOT * N_CORES            # 784 incl. dummy blocks
LAST_VALID = N_NODES - (N_BLK_G - 1) * P     # 32 rows in last global block
BATCH = 32                            # chunks per compute batch
DUMMY_OFF = 1000.0                    # one-hot-killing row offset for pad slots


def _build_program(cap):
    """Emit + compile the single-core SPMD program. cap: [N_SLOT] run
    capacities in edges, each a multiple of 128."""
    import concourse.bacc as bacc
    import concourse.mybir as mybir
    import concourse.tile as tile
    from concourse.masks import make_identity

    f32 = mybir.dt.float32
    bf16 = mybir.dt.bfloat16
    i32 = mybir.dt.int32

    cap = [int(v) for v in cap]
    C = sum(cap) // P
    # chunk -> owning run, with start/stop chunk ids
    chunk_run = []
    run_first = []
    run_last = []
    pos = 0
    for j, cp in enumerate(cap):
        nch = cp // P
        run_first.append(pos)
        run_last.append(pos + nch - 1)
        chunk_run.extend([j] * nch)
        pos += nch
    assert pos == C

    nc = bacc.Bacc("TRN2", target_bir_lowering=False, debug=False)

    xexp_d = nc.dram_tensor("xexp", [P, C * D], bf16, kind="ExternalInput")
    meta_d = nc.dram_tensor("meta", [P, 4 * C], f32, kind="ExternalInput")
    rowoff_d = nc.dram_tensor("rowoff_t", [P, C], bf16, kind="ExternalInput")
    repsh_d = nc.dram_tensor("rep_sh", [P, N_SLOT], f32, kind="ExternalInput")
    xself_d = nc.dram_tensor("x_self", [P, N_SLOT * D], bf16,
                             kind="ExternalInput")
    wcat_d = nc.dram_tensor("w_cat", [2 * D, D], bf16, kind="ExternalInput")
    out_d = nc.dram_tensor("out", [N_SLOT * P, D], f32, kind="ExternalOutput")

    AL = mybir.AluOpType
    ACT = mybir.ActivationFunctionType

    with tile.TileContext(nc) as tc:
        with (
            tc.tile_pool(name="meta", bufs=1) as meta,
            tc.tile_pool(name="gather", bufs=3) as gpool,
            tc.tile_pool(name="work", bufs=3) as wpool,
            tc.tile_pool(name="onehot", bufs=3) as opool,
            tc.tile_pool(name="const", bufs=1) as cpool,
            tc.tile_pool(name="fin", bufs=3) as fpool,
            tc.tile_pool(name="psum", bufs=3, space="PSUM") as psum,
            tc.tile_pool(name="psumT", bufs=2, space="PSUM") as psumT,
        ):
            meta_s = meta.tile([P, 4, C], f32)
            rowoff_s = meta.tile([P, C], bf16)
            repsh_s = meta.tile([P, N_SLOT], f32)
            xself_s = meta.tile([P, N_SLOT, D], bf16)
            coef16 = meta.tile([P, C], bf16)
            wcat_s = cpool.tile([2 * D, D], bf16)
            ident = cpool.tile([P, P], bf16)
            iota_i = cpool.tile([P, P], i32)
            iota1 = cpool.tile([P, P], bf16)
            iota_mat = cpool.tile([P, BATCH, P], bf16)
            nc.sync.dma_start(out=meta_s[:].rearrange("p f c -> p (f c)"),
                              in_=meta_d[:])
            nc.sync.dma_start(out=rowoff_s[:], in_=rowoff_d[:])
            nc.sync.dma_start(out=repsh_s[:], in_=repsh_d[:])
            nc.sync.dma_start(out=xself_s[:].rearrange("p j d -> p (j d)"),
                              in_=xself_d[:])
            nc.sync.dma_start(out=wcat_s[:], in_=wcat_d[:])
            make_identity(nc, ident[:])
            nc.gpsimd.iota(iota_i[:], pattern=[[1, P]], base=0,
                           channel_multiplier=0)
            nc.vector.tensor_copy(out=iota1[:], in_=iota_i[:])
            nc.vector.tensor_copy(
                out=iota_mat[:],
                in_=iota1[:].rearrange("p (b n) -> p b n", b=1)
                    .to_broadcast([P, BATCH, P]))

            # coef for all slots, computed once: sw*sigmoid(rr+rc)*ns
            coef = meta.tile([P, C], f32)
            nc.vector.tensor_tensor(out=coef[:], in0=meta_s[:, 1, :],
                                    in1=meta_s[:, 2, :], op=AL.add)
            nc.scalar.activation(coef[:], coef[:], ACT.Sigmoid)
            nc.vector.tensor_tensor(out=coef[:], in0=coef[:],
                                    in1=meta_s[:, 0, :], op=AL.mult)
            nc.vector.tensor_tensor(out=coef[:], in0=coef[:],
                                    in1=meta_s[:, 3, :], op=AL.mult)
            nc.vector.tensor_copy(out=coef16[:], in_=coef[:])

            run_psum = {}  # slot -> live psum tile for its run

            def finalize_block(j):
                ps = run_psum.pop(j)
                dn = fpool.tile([P, 1], f32, tag="dn")
                nc.any.tensor_scalar_add(out=dn[:], in0=ps[:, D:D + 1],
                                         scalar1=1e-6)
                recip = fpool.tile([P, 1], f32, tag="recip")
                nc.vector.reciprocal(out=recip[:], in_=dn[:])
                srep = fpool.tile([P, 1], f32, tag="srep")
                nc.scalar.activation(srep[:], repsh_s[:, j:j + 1], ACT.Sigmoid)
                cat = fpool.tile([P, 2 * D], bf16, tag="cat")
                nc.scalar.activation(cat[:, 0:D], ps[:, 0:D], ACT.Copy,
                                     scale=recip[:])
                nc.scalar.activation(cat[:, D:2 * D], xself_s[:, j, :],
                                     ACT.Copy, scale=srep[:])
                catT_ps = psumT.tile([P, P], bf16, tag="catT")
                nc.tensor.transpose(out=catT_ps[:], in_=cat[:],
                                    identity=ident[:])
                catT = fpool.tile([P, P], bf16, tag="catT_s")
                nc.vector.tensor_copy(out=catT[:], in_=catT_ps[:])
                out_ps = psumT.tile([P, D], f32, tag="out_ps")
                nc.tensor.matmul(out=out_ps[:], lhsT=catT[:], rhs=wcat_s[:],
                                 start=True, stop=True)
                outb = fpool.tile([P, D], f32, tag="outb")
                lk = fpool.tile([P, D], f32, tag="lk")
                nc.any.tensor_scalar_mul(out=lk[:], in0=out_ps[:], scalar1=0.01)
                nc.any.tensor_tensor(out=outb[:], in0=out_ps[:], in1=lk[:],
                                     op=AL.max)
                nc.sync.dma_start(out=out_d[j * P:(j + 1) * P, :], in_=outb[:])

            for c0 in range(0, C, BATCH):
                nb = min(BATCH, C - c0)
                xg = gpool.tile([P, BATCH * D], bf16, tag="xg")
                nc.sync.dma_start(out=xg[:, :nb * D],
                                  in_=xexp_d[:, c0 * D:(c0 + nb) * D])
                xg_v = xg[:, :nb * D].rearrange("p (b d) -> p b d", d=D)

                xs = wpool.tile([P, BATCH, D + 1], bf16, tag="xs")
                nc.vector.memset(xs[:, :nb, D:D + 1], 1.0)
                nc.vector.tensor_tensor(
                    out=xs[:, :nb, 0:D], in0=xg_v,
                    in1=coef16[:, c0:c0 + nb]
                        .rearrange("p (b o) -> p b o", o=1)
                        .to_broadcast([P, nb, D]),
                    op=AL.mult)

                oh = opool.tile([P, BATCH, P], bf16, tag="oh")
                nc.vector.tensor_tensor(
                    out=oh[:, :nb, :],
                    in0=rowoff_s[:, c0:c0 + nb]
                        .rearrange("p (b o) -> p b o", o=1)
                        .to_broadcast([P, nb, P]),
                    in1=iota_mat[:, :nb, :],
                    op=AL.is_equal)

                for i in range(nb):
                    ci = c0 + i
                    j = chunk_run[ci]
                    is_start = ci == run_first[j]
                    is_stop = ci == run_last[j]
                    if is_start:
                        run_psum[j] = psum.tile([P, D + 1], f32, tag="agg",
                                                name="agg_ps")
                    nc.tensor.matmul(out=run_psum[j][:], lhsT=oh[:, i, :],
                                     rhs=xs[:, i, :],
                                     start=is_start, stop=is_stop)
                    if is_stop:
                        finalize_block(j)

    nc.compile()
    return nc


def _preprocess(x, edge_index, sim_weight, rep, node_signal):
    """Host-side layout: deal destination blocks to cores (snake by count),
    group edges into (core, block-slot) runs padded to 128-edge chunks,
    pre-expand x[col] into slot order (bf16), produce per-core arrays."""
    import ml_dtypes

    bf16 = ml_dtypes.bfloat16
    row = np.ascontiguousarray(edge_index[0]).astype(np.int64)
    col = np.ascontiguousarray(edge_index[1]).astype(np.int64)
    sw = np.ascontiguousarray(sim_weight).astype(np.float32)
    rep_f = np.ascontiguousarray(rep).astype(np.float32)
    ns_f = np.ascontiguousarray(node_signal).astype(np.float32)
    x16 = np.ascontiguousarray(x).astype(bf16)
    E = row.shape[0]

    gb = row >> 7
    off = (row & 127).astype(np.float32)

    counts = np.bincount(gb, minlength=N_BLK_T).astype(np.int64)
    order_desc = np.argsort(-counts, kind="stable")
    assign = np.empty((N_CORES, N_SLOT), dtype=np.int64)
    for j in range(N_SLOT):
        ids = order_desc[j * N_CORES:(j + 1) * N_CORES]
        if j % 2 == 0:
            assign[:, j] = ids
        else:
            assign[::-1, j] = ids
    inv_core = np.empty(N_BLK_T, dtype=np.int64)
    inv_slot = np.empty(N_BLK_T, dtype=np.int64)
    for c in range(N_CORES):
        inv_core[assign[c]] = c
        inv_slot[assign[c]] = np.arange(N_SLOT)

    cap = ((counts[assign].max(axis=0) + P - 1) // P) * P
    cap = np.maximum(cap, P)
    slot_base = np.zeros(N_SLOT + 1, dtype=np.int64)
    np.cumsum(cap, out=slot_base[1:])
    tot_pc = int(slot_base[-1])          # slots per core
    C = tot_pc // P

    core_e = inv_core[gb]
    slot_e = inv_slot[gb]
    key = core_e * N_SLOT + slot_e
    order = np.argsort(key, kind="stable")
    gcounts = np.bincount(key, minlength=N_CORES * N_SLOT)
    gstart = np.zeros(N_CORES * N_SLOT + 1, dtype=np.int64)
    np.cumsum(gcounts, out=gstart[1:])
    ko = key[order]
    rank = np.arange(E, dtype=np.int64) - gstart[ko]
    abs_slot = core_e[order] * tot_pc + slot_base[slot_e[order]] + rank

    tot = N_CORES * tot_pc
    xexp = np.zeros((tot, D), dtype=bf16)
    xexp[abs_slot] = x16[col[order]]
    sw_p = np.zeros(tot, dtype=np.float32)
    rr_p = np.zeros(tot, dtype=np.float32)
    rc_p = np.zeros(tot, dtype=np.float32)
    ns_p = np.zeros(tot, dtype=np.float32)
    rowoff_p = np.full(tot, DUMMY_OFF, dtype=np.float32)
    sw_p[abs_slot] = sw[order]
    rr_p[abs_slot] = rep_f[row[order]]
    rc_p[abs_slot] = rep_f[col[order]]
    ns_p[abs_slot] = ns_f[col[order]]
    rowoff_p[abs_slot] = off[order]

    xexp_t = np.ascontiguousarray(
        xexp.reshape(N_CORES, C, P, D).transpose(0, 2, 1, 3)
        .reshape(N_CORES, P, C * D))

    def per_core(a):
        return a.reshape(N_CORES, C, P).transpose(0, 2, 1)

    meta_t = np.ascontiguousarray(
        np.stack([per_core(sw_p), per_core(rr_p), per_core(rc_p),
                  per_core(ns_p)], axis=2).reshape(N_CORES, P, 4 * C))
    rowoff_t = np.ascontiguousarray(per_core(rowoff_p)).astype(bf16)

    ids = assign[:, :, None] * P + np.arange(P)[None, None, :]  # [8, 98, 128]
    valid = ids < N_NODES
    ids_c = np.minimum(ids, N_NODES - 1)
    xsel = np.where(valid[..., None], x16[ids_c], bf16(0))
    xself_t = np.ascontiguousarray(
        xsel.transpose(0, 2, 1, 3).reshape(N_CORES, P, N_SLOT * D))
    repsh_t = np.ascontiguousarray(
        np.where(valid, rep_f[ids_c], 0.0).transpose(0, 2, 1))

    return cap, assign, xexp_t, meta_t, rowoff_t, repsh_t, xself_t


_compiled = {}


def _get_program(cap):
    key = tuple(cap.tolist())
    if key not in _compiled:
        _compiled[key] = _build_program(cap)
    return _compiled[key]


def run(x, edge_index, sim_weight, rep, node_signal, W, W_self, trace=False):
    from concourse.bass_utils import run_bass_kernel_spmd
    import ml_dtypes

    (cap, assign, xexp_t, meta_t, rowoff_t, repsh_t,
     xself_t) = _preprocess(x, edge_index, sim_weight, rep, node_signal)
    w_cat = np.ascontiguousarray(
        np.concatenate([np.asarray(W, dtype=np.float32),
                        np.asarray(W_self, dtype=np.float32)],
                       axis=0)).astype(ml_dtypes.bfloat16)
    nc = _get_program(cap)
    in_maps = []
    for c in range(N_CORES):
        in_maps.append({
            "xexp": xexp_t[c],
            "meta": meta_t[c],
            "rowoff_t": rowoff_t[c],
            "rep_sh": repsh_t[c],
            "x_self": xself_t[c],
            "w_cat": w_cat,
        })
    res = run_bass_kernel_spmd(nc, in_maps, core_ids=list(range(N_CORES)),
                               trace=trace)
    out = np.empty((N_NODES, D), dtype=np.float32)
    for c in range(N_CORES):
        oc = res.results[c]["out"]
        for j in range(N_SLOT):
            g = int(assign[c][j])
            if g >= N_BLK_G:
                continue
            n = P if g < N_BLK_G - 1 else LAST_VALID
            out[g * P:g * P + n] = oc[j * P:j * P + n]
    return out, res


def kernel(x, edge_index, sim_weight, rep, node_signal, W, W_self):
    out, _ = run(x, edge_index, sim_weight, rep, node_signal, W, W_self)
    return out


# revision 5
# speedup vs baseline: 8.0609x; 1.7197x over previous
"""BehaviorAwareGCNLayer on 8 Trainium2 NeuronCores.

Math (reference):
    hx  = x @ W
    out[r] = (1/deg[r]) * sum_{e: row[e]=r} sim_w[e]*sigmoid(rep[row]+rep[col])*ns[col] * hx[col]
    out += sigmoid(rep) * (x @ W_self);  leaky_relu(out, 0.01)

Device strategy (destination sharding, no collectives):
  - By linearity, W is applied AFTER aggregation: agg[r] = sum coef_e * x[col_e],
    out[r] = (agg[r]/deg[r]) @ W + sigmoid(rep_r)*(x_r @ W_self).
  - Host does LAYOUT only (grouping/padding/fancy-index copies, one-hot
    position encoding, edge-count metadata); all value math (sigmoid,
    products, sums, matmuls) happens on device.
  - Destination rows are grouped into 32-row blocks (3125 of them); blocks
    are dealt to the 8 cores snake-wise by descending edge count, so
    per-slot capacities are nearly equal across cores -> single SPMD
    program, minimal padding. 392 block slots per core; slots 4t..4t+3
    share one [128, 64] PSUM tile (block q occupies partitions 32q..32q+31
    via the PE column-group feature).
  - Edges are grouped per (core, slot) into runs padded to whole 128-edge
    chunks, so every chunk belongs to exactly one 32-row block.
  - The host pre-expands x[col_e] (bf16) AND the 32-wide destination one-hot
    (fp8, a pure position encoding of row_e & 31) into edge-slot order; the
    device STREAMS both with plain contiguous DMA (no gather ucode at all).
  - coef = sw*sigmoid(rep_row+rep_col)*ns_col for all slots is computed once
    up front from 4 streamed f32 metadata planes; per chunk the one-hot is
    scaled by coef (one DVE op) and used as the matmul lhsT against the raw
    streamed x rows: psum[32q+j, :] += sum_e coef_e*onehot[e,j]*x[col_e].
  - deg is shipped from host (edge bincount, clamped to >=1 -- layout
    metadata); finalize: cat = [agg*(1/deg) | sigmoid(rep)*x_self] in bf16
    (ACT engine), one PE transpose + one matmul with [W; W_self] applies
    both weights, leaky_relu, DMA out. Output rows are re-assembled on host
    per the block permutation.
"""
import sys

if "/opt/trn_rl_repo" not in sys.path:
    sys.path.insert(0, "/opt/trn_rl_repo")

import numpy as np

P = 128
D = 64
B32 = 32                              # dest block rows
N_NODES = 100000
N_CORES = 8
N_BLK32 = N_NODES // B32              # 3125 global 32-row blocks (exact)
N_SLOT = 392                          # block slots per core (392*8=3136)
N_BLK_T = N_SLOT * N_CORES
N_GRP = N_SLOT // 4                   # 98 psum quad groups per core
BATCH = 32                            # chunks per compute batch
USE_FP8_ONEHOT = True


def _build_program(cap):
    """Emit + compile the single-core SPMD program. cap: [N_SLOT] run
    capacities in edges, each a multiple of 128."""
    import concourse.bacc as bacc
    import concourse.mybir as mybir
    import concourse.tile as tile
    from concourse.masks import make_identity

    f32 = mybir.dt.float32
    bf16 = mybir.dt.bfloat16
    oh_dt = mybir.dt.float8e4 if USE_FP8_ONEHOT else bf16

    cap = [int(v) for v in cap]
    C = sum(cap) // P
    chunk_slot = []
    run_first = []
    run_last = []
    pos = 0
    for j, cp in enumerate(cap):
        nch = cp // P
        run_first.append(pos)
        run_last.append(pos + nch - 1)
        chunk_slot.extend([j] * nch)
        pos += nch
    assert pos == C

    nc = bacc.Bacc("TRN2", target_bir_lowering=False, debug=False)

    xexp_d = nc.dram_tensor("xexp", [P, C * D], bf16, kind="ExternalInput")
    oh_d = nc.dram_tensor("oh", [P, C * B32], oh_dt, kind="ExternalInput")
    meta_d = nc.dram_tensor("meta", [P, 4 * C], f32, kind="ExternalInput")
    deg_d = nc.dram_tensor("deg", [P, N_GRP], f32, kind="ExternalInput")
    repsh_d = nc.dram_tensor("rep_sh", [P, N_GRP], f32, kind="ExternalInput")
    xself_d = nc.dram_tensor("x_self", [P, N_GRP * D], bf16,
                             kind="ExternalInput")
    wcat_d = nc.dram_tensor("w_cat", [2 * D, D], bf16, kind="ExternalInput")
    out_d = nc.dram_tensor("out", [N_GRP * P, D], f32, kind="ExternalOutput")

    AL = mybir.AluOpType
    ACT = mybir.ActivationFunctionType

    with tile.TileContext(nc) as tc:
        with (
            tc.tile_pool(name="meta", bufs=1) as meta,
            tc.tile_pool(name="gather", bufs=3) as gpool,
            tc.tile_pool(name="onehot", bufs=3) as opool,
            tc.tile_pool(name="const", bufs=1) as cpool,
            tc.tile_pool(name="fin", bufs=3) as fpool,
            tc.tile_pool(name="psum", bufs=3, space="PSUM") as psum,
            tc.tile_pool(name="psumT", bufs=2, space="PSUM") as psumT,
        ):
            meta_s = meta.tile([P, 4, C], f32)
            deg_s = meta.tile([P, N_GRP], f32)
            repsh_s = meta.tile([P, N_GRP], f32)
            xself_s = meta.tile([P, N_GRP, D], bf16)
            coef16 = meta.tile([P, C], bf16)
            wcat_s = cpool.tile([2 * D, D], bf16)
            ident = cpool.tile([P, P], bf16)
            nc.sync.dma_start(out=meta_s[:].rearrange("p f c -> p (f c)"),
                              in_=meta_d[:])
            nc.sync.dma_start(out=deg_s[:], in_=deg_d[:])
            nc.sync.dma_start(out=repsh_s[:], in_=repsh_d[:])
            nc.sync.dma_start(out=xself_s[:].rearrange("p j d -> p (j d)"),
                              in_=xself_d[:])
            nc.sync.dma_start(out=wcat_s[:], in_=wcat_d[:])
            make_identity(nc, ident[:])

            # coef for all slots, computed once: sw*sigmoid(rr+rc)*ns
            coef = meta.tile([P, C], f32)
            nc.vector.tensor_tensor(out=coef[:], in0=meta_s[:, 1, :],
                                    in1=meta_s[:, 2, :], op=AL.add)
            nc.scalar.activation(coef[:], coef[:], ACT.Sigmoid)
            nc.vector.tensor_tensor(out=coef[:], in0=coef[:],
                                    in1=meta_s[:, 0, :], op=AL.mult)
            nc.vector.tensor_tensor(out=coef[:], in0=coef[:],
                                    in1=meta_s[:, 3, :], op=AL.mult)
            nc.vector.tensor_copy(out=coef16[:], in_=coef[:])

            run_ps = {}  # group -> live psum tile

            def finalize_group(t):
                ps = run_ps.pop(t)
                recip = fpool.tile([P, 1], f32, tag="recip")
                nc.vector.reciprocal(out=recip[:], in_=deg_s[:, t:t + 1])
                srep = fpool.tile([P, 1], f32, tag="srep")
                nc.scalar.activation(srep[:], repsh_s[:, t:t + 1], ACT.Sigmoid)
                cat = fpool.tile([P, 2 * D], bf16, tag="cat")
                nc.scalar.activation(cat[:, 0:D], ps[:], ACT.Copy,
                                     scale=recip[:])
                nc.scalar.activation(cat[:, D:2 * D], xself_s[:, t, :],
                                     ACT.Copy, scale=srep[:])
                catT_ps = psumT.tile([P, P], bf16, tag="catT")
                nc.tensor.transpose(out=catT_ps[:], in_=cat[:],
                                    identity=ident[:])
                catT = fpool.tile([P, P], bf16, tag="catT_s")
                nc.vector.tensor_copy(out=catT[:], in_=catT_ps[:])
                out_ps = psumT.tile([P, D], f32, tag="out_ps")
                nc.tensor.matmul(out=out_ps[:], lhsT=catT[:], rhs=wcat_s[:],
                                 start=True, stop=True)
                outb = fpool.tile([P, D], f32, tag="outb")
                lk = fpool.tile([P, D], f32, tag="lk")
                nc.any.tensor_scalar_mul(out=lk[:], in0=out_ps[:], scalar1=0.01)
                nc.any.tensor_tensor(out=outb[:], in0=out_ps[:], in1=lk[:],
                                     op=AL.max)
                nc.sync.dma_start(out=out_d[t * P:(t + 1) * P, :], in_=outb[:])

            for c0 in range(0, C, BATCH):
                nb = min(BATCH, C - c0)
                xg = gpool.tile([P, BATCH * D], bf16, tag="xg")
                nc.sync.dma_start(out=xg[:, :nb * D],
                                  in_=xexp_d[:, c0 * D:(c0 + nb) * D])
                xg_v = xg[:, :nb * D].rearrange("p (b d) -> p b d", d=D)

                oh8 = opool.tile([P, BATCH * B32], oh_dt, tag="oh8")
                nc.sync.dma_start(out=oh8[:, :nb * B32],
                                  in_=oh_d[:, c0 * B32:(c0 + nb) * B32])
                oh8_v = oh8[:, :nb * B32].rearrange("p (b n) -> p b n", n=B32)

                ohw = opool.tile([P, BATCH, B32], bf16, tag="ohw")
                nc.vector.tensor_tensor(
                    out=ohw[:, :nb, :], in0=oh8_v,
                    in1=coef16[:, c0:c0 + nb]
                        .rearrange("p (b o) -> p b o", o=1)
                        .to_broadcast([P, nb, B32]),
                    op=AL.mult)

                for i in range(nb):
                    ci = c0 + i
                    j = chunk_slot[ci]
                    t, q = j >> 2, j & 3
                    is_start = ci == run_first[j]
                    is_stop = ci == run_last[j]
                    if is_start and q == 0:
                        run_ps[t] = psum.tile([P, D], f32, tag="agg",
                                              name="agg_ps")
                    nc.tensor.matmul(
                        out=run_ps[t][q * B32:(q + 1) * B32, :],
                        lhsT=ohw[:, i, :], rhs=xg_v[:, i, :],
                        start=is_start, stop=is_stop,
                        tile_position=(0, q * B32))
                    if is_stop and q == 3:
                        finalize_group(t)

    nc.compile()
    return nc


def _preprocess(x, edge_index, sim_weight, rep, node_signal):
    """Host-side layout: deal destination 32-row blocks to cores (snake by
    count), group edges into (core, slot) runs padded to 128-edge chunks,
    pre-expand x[col] (bf16) and the destination one-hot (fp8) into slot
    order, produce per-core arrays + deg/rep/x_self in psum-partition
    layout."""
    import ml_dtypes

    bf16 = ml_dtypes.bfloat16
    oh_np = ml_dtypes.float8_e4m3fn if USE_FP8_ONEHOT else bf16
    row = np.ascontiguousarray(edge_index[0]).astype(np.int64)
    col = np.ascontiguousarray(edge_index[1]).astype(np.int64)
    sw = np.ascontiguousarray(sim_weight).astype(np.float32)
    rep_f = np.ascontiguousarray(rep).astype(np.float32)
    ns_f = np.ascontiguousarray(node_signal).astype(np.float32)
    x16 = np.ascontiguousarray(x).astype(bf16)
    E = row.shape[0]

    gb = row >> 5
    off = (row & 31).astype(np.int64)

    counts = np.bincount(gb, minlength=N_BLK_T).astype(np.int64)
    order_desc = np.argsort(-counts, kind="stable")
    assign = np.empty((N_CORES, N_SLOT), dtype=np.int64)
    for j in range(N_SLOT):
        ids = order_desc[j * N_CORES:(j + 1) * N_CORES]
        if j % 2 == 0:
            assign[:, j] = ids
        else:
            assign[::-1, j] = ids
    inv_core = np.empty(N_BLK_T, dtype=np.int64)
    inv_slot = np.empty(N_BLK_T, dtype=np.int64)
    for c in range(N_CORES):
        inv_core[assign[c]] = c
        inv_slot[assign[c]] = np.arange(N_SLOT)

    cap = ((counts[assign].max(axis=0) + P - 1) // P) * P
    cap = np.maximum(cap, P)
    slot_base = np.zeros(N_SLOT + 1, dtype=np.int64)
    np.cumsum(cap, out=slot_base[1:])
    tot_pc = int(slot_base[-1])
    C = tot_pc // P

    core_e = inv_core[gb]
    slot_e = inv_slot[gb]
    key = core_e * N_SLOT + slot_e
    order = np.argsort(key, kind="stable")
    gcounts = np.bincount(key, minlength=N_CORES * N_SLOT)
    gstart = np.zeros(N_CORES * N_SLOT + 1, dtype=np.int64)
    np.cumsum(gcounts, out=gstart[1:])
    rank = np.arange(E, dtype=np.int64) - gstart[key[order]]
    abs_slot = core_e[order] * tot_pc + slot_base[slot_e[order]] + rank

    tot = N_CORES * tot_pc
    xexp = np.zeros((tot, D), dtype=bf16)
    xexp[abs_slot] = x16[col[order]]
    ohx = np.zeros((tot, B32), dtype=oh_np)
    ohx[abs_slot, off[order]] = oh_np(1.0)
    sw_p = np.zeros(tot, dtype=np.float32)
    rr_p = np.zeros(tot, dtype=np.float32)
    rc_p = np.zeros(tot, dtype=np.float32)
    ns_p = np.zeros(tot, dtype=np.float32)
    sw_p[abs_slot] = sw[order]
    rr_p[abs_slot] = rep_f[row[order]]
    rc_p[abs_slot] = rep_f[col[order]]
    ns_p[abs_slot] = ns_f[col[order]]

    xexp_t = np.ascontiguousarray(
        xexp.reshape(N_CORES, C, P, D).transpose(0, 2, 1, 3)
        .reshape(N_CORES, P, C * D))
    oh_t = np.ascontiguousarray(
        ohx.reshape(N_CORES, C, P, B32).transpose(0, 2, 1, 3)
        .reshape(N_CORES, P, C * B32))

    def per_core(a):
        return a.reshape(N_CORES, C, P).transpose(0, 2, 1)

    meta_t = np.ascontiguousarray(
        np.stack([per_core(sw_p), per_core(rr_p), per_core(rc_p),
                  per_core(ns_p)], axis=2).reshape(N_CORES, P, 4 * C))

    # psum-partition layout grids: row_id(c, p, t) for partition p, group t
    pj = np.arange(P) // B32                 # quad index within group
    po = np.arange(P) % B32                  # offset within 32-block
    slot_grid = (np.arange(N_GRP)[None, :] * 4 + pj[:, None])   # [P, N_GRP]
    gb_grid = assign[:, slot_grid]                              # [8, P, N_GRP]
    rid = gb_grid * B32 + po[None, :, None]                     # [8, P, N_GRP]
    valid = gb_grid < N_BLK32
    rid_c = np.minimum(rid, N_NODES - 1)

    degc = np.maximum(np.bincount(row, minlength=N_NODES), 1).astype(
        np.float32)
    deg_t = np.ascontiguousarray(np.where(valid, degc[rid_c], 1.0))
    repsh_t = np.ascontiguousarray(np.where(valid, rep_f[rid_c], 0.0))
    xself_t = np.ascontiguousarray(
        np.where(valid[..., None], x16[rid_c], bf16(0))
        .reshape(N_CORES, P, N_GRP * D))

    return (cap, rid, valid, xexp_t, oh_t, meta_t, deg_t, repsh_t, xself_t)


_compiled = {}


def _get_program(cap):
    key = tuple(cap.tolist())
    if key not in _compiled:
        _compiled[key] = _build_program(cap)
    return _compiled[key]


def run(x, edge_index, sim_weight, rep, node_signal, W, W_self, trace=False):
    from concourse.bass_utils import run_bass_kernel_spmd
    import ml_dtypes

    (cap, rid, valid, xexp_t, oh_t, meta_t, deg_t, repsh_t,
     xself_t) = _preprocess(x, edge_index, sim_weight, rep, node_signal)
    w_cat = np.ascontiguousarray(
        np.concatenate([np.asarray(W, dtype=np.float32),
                        np.asarray(W_self, dtype=np.float32)],
                       axis=0)).astype(ml_dtypes.bfloat16)
    nc = _get_program(cap)
    in_maps = []
    for c in range(N_CORES):
        in_maps.append({
            "xexp": xexp_t[c],
            "oh": oh_t[c],
            "meta": meta_t[c],
            "deg": deg_t[c],
            "rep_sh": repsh_t[c],
            "x_self": xself_t[c],
            "w_cat": w_cat,
        })
    res = run_bass_kernel_spmd(nc, in_maps, core_ids=list(range(N_CORES)),
                               trace=trace)
    out = np.empty((N_NODES, D), dtype=np.float32)
    for c in range(N_CORES):
        oc = res.results[c]["out"]                 # [N_GRP*P, D]
        ocv = oc.reshape(N_GRP, P, D).transpose(1, 0, 2)  # [P, N_GRP, D]
        out[rid[c][valid[c]]] = ocv[valid[c]]
    return out, res


def kernel(x, edge_index, sim_weight, rep, node_signal, W, W_self):
    out, _ = run(x, edge_index, sim_weight, rep, node_signal, W, W_self)
    return out


# revision 7
# speedup vs baseline: 10.4331x; 1.2943x over previous
"""BehaviorAwareGCNLayer on 8 Trainium2 NeuronCores.

Math (reference):
    hx  = x @ W
    out[r] = (1/deg[r]) * sum_{e: row[e]=r} sim_w[e]*sigmoid(rep[row]+rep[col])*ns[col] * hx[col]
    out += sigmoid(rep) * (x @ W_self);  leaky_relu(out, 0.01)

Device strategy (destination sharding, no collectives):
  - By linearity, W is applied AFTER aggregation: agg[r] = sum coef_e * x[col_e],
    out[r] = (agg[r]/deg[r]) @ W + sigmoid(rep_r)*(x_r @ W_self).
  - Host does LAYOUT only (grouping/padding/fancy-index copies, one-hot
    position encoding, edge-count metadata); all value math (sigmoid,
    products, sums, matmuls) happens on device.
  - Destination rows are grouped into 32-row blocks (3125 of them); blocks
    are dealt to the 8 cores snake-wise by descending edge count, so
    per-slot capacities are nearly equal across cores -> single SPMD
    program, minimal padding. 392 block slots per core; slots 4t..4t+3
    share one [128, 64] PSUM tile (block q occupies partitions 32q..32q+31
    via the PE column-group feature).
  - Edges are grouped per (core, slot) into runs padded to whole 128-edge
    chunks, so every chunk belongs to exactly one 32-row block.
  - The host pre-expands x[col_e] (bf16) AND the 32-wide destination one-hot
    (fp8, a pure position encoding of row_e & 31) into edge-slot order, in
    batch-contiguous DRAM layout; the device STREAMS both with plain
    contiguous DMA on separate engine queues (no gather ucode at all).
  - coef = sw*sigmoid(rep_row+rep_col)*ns_col for all slots is computed once
    up front from 4 streamed bf16 metadata planes; per chunk the one-hot is
    scaled by coef (one DVE op) and used as the matmul lhsT against the raw
    streamed x rows: psum[32q+j, :] += sum_e coef_e*onehot[e,j]*x[col_e].
  - deg is shipped from host (edge bincount clamped to >=1 -- layout
    metadata); 1/deg, sigmoid(rep), and sigmoid(rep)*x_self are computed
    once up front. Finalize per quad group: one ACT copy (agg/deg -> bf16),
    two PE transposes build [agg/deg | srep*x_self]^T directly in PSUM, one
    matmul with [W; W_self] applies both weights, leaky_relu on ACT, DMA out.
    Output rows are re-assembled on host per the block permutation.
"""
import sys

if "/opt/trn_rl_repo" not in sys.path:
    sys.path.insert(0, "/opt/trn_rl_repo")

import numpy as np

P = 128
D = 64
B32 = 32                              # dest block rows
N_NODES = 100000
N_CORES = 8
N_BLK32 = N_NODES // B32              # 3125 global 32-row blocks (exact)
N_SLOT = 392                          # block slots per core (392*8=3136)
N_BLK_T = N_SLOT * N_CORES
N_GRP = N_SLOT // 4                   # 98 psum quad groups per core
BATCH = 64                            # chunks per compute batch
USE_FP8_ONEHOT = True
USE_ACT_LRELU = True


def _build_program(cap):
    """Emit + compile the single-core SPMD program. cap: [N_SLOT] run
    capacities in edges, each a multiple of 128; sum(cap) is a multiple of
    128*BATCH."""
    import concourse.bacc as bacc
    import concourse.mybir as mybir
    import concourse.tile as tile
    from concourse.masks import make_identity

    f32 = mybir.dt.float32
    bf16 = mybir.dt.bfloat16
    oh_dt = mybir.dt.float8e4 if USE_FP8_ONEHOT else bf16

    cap = [int(v) for v in cap]
    C = sum(cap) // P
    assert C % BATCH == 0
    NB = C // BATCH
    chunk_slot = []
    run_first = []
    run_last = []
    pos = 0
    for j, cp in enumerate(cap):
        nch = cp // P
        run_first.append(pos)
        run_last.append(pos + nch - 1)
        chunk_slot.extend([j] * nch)
        pos += nch
    assert pos == C

    nc = bacc.Bacc("TRN2", target_bir_lowering=False, debug=False)

    xexp_d = nc.dram_tensor("xexp", [NB * P, BATCH * D], bf16,
                            kind="ExternalInput")
    oh_d = nc.dram_tensor("oh", [NB * P, BATCH * B32], oh_dt,
                          kind="ExternalInput")
    meta_d = nc.dram_tensor("meta", [P, 4 * C], bf16, kind="ExternalInput")
    deg_d = nc.dram_tensor("deg", [P, N_GRP], f32, kind="ExternalInput")
    repsh_d = nc.dram_tensor("rep_sh", [P, N_GRP], f32, kind="ExternalInput")
    xself_d = nc.dram_tensor("x_self", [P, N_GRP * D], bf16,
                             kind="ExternalInput")
    wcat_d = nc.dram_tensor("w_cat", [2 * D, D], bf16, kind="ExternalInput")
    out_d = nc.dram_tensor("out", [N_GRP * P, D], f32, kind="ExternalOutput")

    AL = mybir.AluOpType
    ACT = mybir.ActivationFunctionType

    with tile.TileContext(nc) as tc:
        with (
            tc.tile_pool(name="meta", bufs=1) as meta,
            tc.tile_pool(name="gather", bufs=3) as gpool,
            tc.tile_pool(name="onehot", bufs=3) as opool,
            tc.tile_pool(name="const", bufs=1) as cpool,
            tc.tile_pool(name="fin", bufs=4) as fpool,
            tc.tile_pool(name="psum", bufs=4, space="PSUM") as psum,
            tc.tile_pool(name="psumT", bufs=2, space="PSUM") as psumT,
        ):
            meta_s = meta.tile([P, 4, C], bf16)
            deg_s = meta.tile([P, N_GRP], f32)
            repsh_s = meta.tile([P, N_GRP], f32)
            xself_s = meta.tile([P, N_GRP, D], bf16)
            coef16 = meta.tile([P, C], bf16)
            recip_all = meta.tile([P, N_GRP], f32)
            srep_all = meta.tile([P, N_GRP], f32)
            xselfS = meta.tile([P, N_GRP, D], bf16)
            wcat_s = cpool.tile([2 * D, D], bf16)
            ident = cpool.tile([P, P], bf16)
            nc.sync.dma_start(out=meta_s[:].rearrange("p f c -> p (f c)"),
                              in_=meta_d[:])
            nc.gpsimd.dma_start(out=deg_s[:], in_=deg_d[:])
            nc.gpsimd.dma_start(out=repsh_s[:], in_=repsh_d[:])
            nc.scalar.dma_start(out=xself_s[:].rearrange("p j d -> p (j d)"),
                                in_=xself_d[:])
            nc.scalar.dma_start(out=wcat_s[:], in_=wcat_d[:])
            make_identity(nc, ident[:])

            # one-time prep: coef, 1/deg, sigmoid(rep), srep*x_self
            coef = meta.tile([P, C], f32)
            nc.vector.tensor_tensor(out=coef[:], in0=meta_s[:, 1, :],
                                    in1=meta_s[:, 2, :], op=AL.add)
            nc.scalar.activation(coef[:], coef[:], ACT.Sigmoid)
            nc.vector.tensor_tensor(out=coef[:], in0=coef[:],
                                    in1=meta_s[:, 0, :], op=AL.mult)
            nc.vector.tensor_tensor(out=coef16[:], in0=coef[:],
                                    in1=meta_s[:, 3, :], op=AL.mult)
            nc.vector.reciprocal(out=recip_all[:], in_=deg_s[:])
            nc.scalar.activation(srep_all[:], repsh_s[:], ACT.Sigmoid)
            nc.vector.tensor_tensor(
                out=xselfS[:], in0=xself_s[:],
                in1=srep_all[:].rearrange("p (j o) -> p j o", o=1)
                    .to_broadcast([P, N_GRP, D]),
                op=AL.mult)

            run_ps = {}  # group -> live psum tile

            def finalize_group(t):
                ps = run_ps.pop(t)
                cat0 = fpool.tile([P, D], bf16, tag="cat0")
                nc.scalar.activation(cat0[:], ps[:], ACT.Copy,
                                     scale=recip_all[:, t:t + 1])
                catT_ps = psumT.tile([P, P], bf16, tag="catT")
                nc.tensor.transpose(out=catT_ps[0:D, :], in_=cat0[:],
                                    identity=ident[:])
                nc.tensor.transpose(out=catT_ps[D:2 * D, :],
                                    in_=xselfS[:, t, :], identity=ident[:],
                                    tile_position=(0, D))
                catT = fpool.tile([P, P], bf16, tag="catT_s")
                nc.vector.tensor_copy(out=catT[:], in_=catT_ps[:])
                out_ps = psumT.tile([P, D], f32, tag="out_ps")
                nc.tensor.matmul(out=out_ps[:], lhsT=catT[:], rhs=wcat_s[:],
                                 start=True, stop=True)
                outb = fpool.tile([P, D], f32, tag="outb")
                if USE_ACT_LRELU:
                    nc.scalar.activation(outb[:], out_ps[:], ACT.Lrelu,
                                         alpha=0.01)
                else:
                    lk = fpool.tile([P, D], f32, tag="lk")
                    nc.any.tensor_scalar_mul(out=lk[:], in0=out_ps[:],
                                             scalar1=0.01)
                    nc.any.tensor_tensor(out=outb[:], in0=out_ps[:],
                                         in1=lk[:], op=AL.max)
                nc.gpsimd.dma_start(out=out_d[t * P:(t + 1) * P, :],
                                    in_=outb[:])

            for b in range(NB):
                c0 = b * BATCH
                xg = gpool.tile([P, BATCH * D], bf16, tag="xg")
                nc.sync.dma_start(out=xg[:],
                                  in_=xexp_d[b * P:(b + 1) * P, :])
                xg_v = xg[:].rearrange("p (b d) -> p b d", d=D)

                oh8 = opool.tile([P, BATCH * B32], oh_dt, tag="oh8")
                nc.scalar.dma_start(out=oh8[:],
                                    in_=oh_d[b * P:(b + 1) * P, :])
                oh8_v = oh8[:].rearrange("p (b n) -> p b n", n=B32)

                ohw = opool.tile([P, BATCH, B32], bf16, tag="ohw")
                nc.vector.tensor_tensor(
                    out=ohw[:], in0=oh8_v,
                    in1=coef16[:, c0:c0 + BATCH]
                        .rearrange("p (b o) -> p b o", o=1)
                        .to_broadcast([P, BATCH, B32]),
                    op=AL.mult)

                for i in range(BATCH):
                    ci = c0 + i
                    j = chunk_slot[ci]
                    t, q = j >> 2, j & 3
                    is_start = ci == run_first[j]
                    is_stop = ci == run_last[j]
                    if is_start and q == 0:
                        run_ps[t] = psum.tile([P, D], f32, tag="agg",
                                              name="agg_ps")
                    nc.tensor.matmul(
                        out=run_ps[t][q * B32:(q + 1) * B32, :],
                        lhsT=ohw[:, i, :], rhs=xg_v[:, i, :],
                        start=is_start, stop=is_stop,
                        tile_position=(0, q * B32))
                    if is_stop and q == 3:
                        finalize_group(t)

    nc.compile()
    return nc


def _preprocess(x, edge_index, sim_weight, rep, node_signal):
    """Host-side layout: deal destination 32-row blocks to cores (snake by
    count), group edges into (core, slot) runs padded to 128-edge chunks,
    pre-expand x[col] (bf16) and the destination one-hot (fp8) into slot
    order, produce per-core arrays + deg/rep/x_self in psum-partition
    layout."""
    import ml_dtypes

    bf16 = ml_dtypes.bfloat16
    oh_np = ml_dtypes.float8_e4m3fn if USE_FP8_ONEHOT else bf16
    row = np.ascontiguousarray(edge_index[0]).astype(np.int64)
    col = np.ascontiguousarray(edge_index[1]).astype(np.int64)
    sw = np.ascontiguousarray(sim_weight).astype(np.float32)
    rep_f = np.ascontiguousarray(rep).astype(np.float32)
    ns_f = np.ascontiguousarray(node_signal).astype(np.float32)
    x16 = np.ascontiguousarray(x).astype(bf16)
    E = row.shape[0]

    gb = row >> 5
    off = (row & 31).astype(np.int64)

    counts = np.bincount(gb, minlength=N_BLK_T).astype(np.int64)
    order_desc = np.argsort(-counts, kind="stable")
    assign = np.empty((N_CORES, N_SLOT), dtype=np.int64)
    for j in range(N_SLOT):
        ids = order_desc[j * N_CORES:(j + 1) * N_CORES]
        if j % 2 == 0:
            assign[:, j] = ids
        else:
            assign[::-1, j] = ids
    inv_core = np.empty(N_BLK_T, dtype=np.int64)
    inv_slot = np.empty(N_BLK_T, dtype=np.int64)
    for c in range(N_CORES):
        inv_core[assign[c]] = c
        inv_slot[assign[c]] = np.arange(N_SLOT)

    cap = ((counts[assign].max(axis=0) + P - 1) // P) * P
    cap = np.maximum(cap, P)
    # pad total capacity to a whole number of BATCH-chunk batches
    capsum = int(cap.sum())
    pad = (-capsum) % (P * BATCH)
    cap[N_SLOT - 1] += pad
    slot_base = np.zeros(N_SLOT + 1, dtype=np.int64)
    np.cumsum(cap, out=slot_base[1:])
    tot_pc = int(slot_base[-1])
    C = tot_pc // P

    core_e = inv_core[gb]
    slot_e = inv_slot[gb]
    key = core_e * N_SLOT + slot_e
    order = np.argsort(key, kind="stable")
    gcounts = np.bincount(key, minlength=N_CORES * N_SLOT)
    gstart = np.zeros(N_CORES * N_SLOT + 1, dtype=np.int64)
    np.cumsum(gcounts, out=gstart[1:])
    rank = np.arange(E, dtype=np.int64) - gstart[key[order]]
    abs_slot = core_e[order] * tot_pc + slot_base[slot_e[order]] + rank

    tot = N_CORES * tot_pc
    xexp = np.zeros((tot, D), dtype=bf16)
    xexp[abs_slot] = x16[col[order]]
    ohx = np.zeros((tot, B32), dtype=oh_np)
    ohx[abs_slot, off[order]] = oh_np(1.0)
    sw_p = np.zeros(tot, dtype=np.float32)
    rr_p = np.zeros(tot, dtype=np.float32)
    rc_p = np.zeros(tot, dtype=np.float32)
    ns_p = np.zeros(tot, dtype=np.float32)
    sw_p[abs_slot] = sw[order]
    rr_p[abs_slot] = rep_f[row[order]]
    rc_p[abs_slot] = rep_f[col[order]]
    ns_p[abs_slot] = ns_f[col[order]]

    NB = C // BATCH
    xexp_t = np.ascontiguousarray(
        xexp.reshape(N_CORES, NB, BATCH, P, D).transpose(0, 1, 3, 2, 4)
        .reshape(N_CORES, NB * P, BATCH * D))
    oh_t = np.ascontiguousarray(
        ohx.reshape(N_CORES, NB, BATCH, P, B32).transpose(0, 1, 3, 2, 4)
        .reshape(N_CORES, NB * P, BATCH * B32))

    def per_core(a):
        return a.reshape(N_CORES, C, P).transpose(0, 2, 1)

    meta_t = np.ascontiguousarray(
        np.stack([per_core(sw_p), per_core(rr_p), per_core(rc_p),
                  per_core(ns_p)], axis=2).reshape(N_CORES, P, 4 * C)
        ).astype(bf16)

    # psum-partition layout grids: row_id(c, p, t) for partition p, group t
    pj = np.arange(P) // B32
    po = np.arange(P) % B32
    slot_grid = (np.arange(N_GRP)[None, :] * 4 + pj[:, None])    # [P, N_GRP]
    gb_grid = assign[:, slot_grid]                               # [8, P, N_GRP]
    rid = gb_grid * B32 + po[None, :, None]                      # [8, P, N_GRP]
    valid = gb_grid < N_BLK32
    rid_c = np.minimum(rid, N_NODES - 1)

    degc = np.maximum(np.bincount(row, minlength=N_NODES), 1).astype(
        np.float32)
    deg_t = np.ascontiguousarray(np.where(valid, degc[rid_c], 1.0))
    repsh_t = np.ascontiguousarray(np.where(valid, rep_f[rid_c], 0.0))
    xself_t = np.ascontiguousarray(
        np.where(valid[..., None], x16[rid_c], bf16(0))
        .reshape(N_CORES, P, N_GRP * D))

    return (cap, rid, valid, xexp_t, oh_t, meta_t, deg_t, repsh_t, xself_t)


_compiled = {}


def _get_program(cap):
    key = tuple(cap.tolist())
    if key not in _compiled:
        _compiled[key] = _build_program(cap)
    return _compiled[key]


def run(x, edge_index, sim_weight, rep, node_signal, W, W_self, trace=False):
    from concourse.bass_utils import run_bass_kernel_spmd
    import ml_dtypes

    (cap, rid, valid, xexp_t, oh_t, meta_t, deg_t, repsh_t,
     xself_t) = _preprocess(x, edge_index, sim_weight, rep, node_signal)
    w_cat = np.ascontiguousarray(
        np.concatenate([np.asarray(W, dtype=np.float32),
                        np.asarray(W_self, dtype=np.float32)],
                       axis=0)).astype(ml_dtypes.bfloat16)
    nc = _get_program(cap)
    in_maps = []
    for c in range(N_CORES):
        in_maps.append({
            "xexp": xexp_t[c],
            "oh": oh_t[c],
            "meta": meta_t[c],
            "deg": deg_t[c],
            "rep_sh": repsh_t[c],
            "x_self": xself_t[c],
            "w_cat": w_cat,
        })
    res = run_bass_kernel_spmd(nc, in_maps, core_ids=list(range(N_CORES)),
                               trace=trace)
    out = np.empty((N_NODES, D), dtype=np.float32)
    for c in range(N_CORES):
        oc = res.results[c]["out"]                 # [N_GRP*P, D]
        ocv = oc.reshape(N_GRP, P, D).transpose(1, 0, 2)  # [P, N_GRP, D]
        out[rid[c][valid[c]]] = ocv[valid[c]]
    return out, res


def kernel(x, edge_index, sim_weight, rep, node_signal, W, W_self):
    out, _ = run(x, edge_index, sim_weight, rep, node_signal, W, W_self)
    return out


# revision 11
# speedup vs baseline: 10.7129x; 1.0268x over previous
"""BehaviorAwareGCNLayer on 8 Trainium2 NeuronCores.

Math (reference):
    hx  = x @ W
    out[r] = (1/deg[r]) * sum_{e: row[e]=r} sim_w[e]*sigmoid(rep[row]+rep[col])*ns[col] * hx[col]
    out += sigmoid(rep) * (x @ W_self);  leaky_relu(out, 0.01)

Device strategy (destination sharding, no collectives):
  - By linearity, W is applied AFTER aggregation: agg[r] = sum coef_e * x[col_e],
    out[r] = (agg[r]/deg[r]) @ W + sigmoid(rep_r)*(x_r @ W_self).
  - Host does LAYOUT only (grouping/padding/fancy-index copies, one-hot
    position encoding, edge-count metadata); all value math (sigmoid,
    products, sums, matmuls) happens on device.
  - Destination rows are grouped into 32-row blocks (3125 of them); blocks
    are dealt to the 8 cores snake-wise by descending edge count, so
    per-slot capacities are nearly equal across cores -> single SPMD
    program, minimal padding. 392 block slots per core; slots 4t..4t+3
    share one [128, 64] PSUM tile (block q occupies partitions 32q..32q+31
    via the PE column-group feature).
  - Edges are grouped per (core, slot) into runs padded to whole 128-edge
    chunks, so every chunk belongs to exactly one 32-row block.
  - The host pre-expands x[col_e] (bf16) AND the 32-wide destination one-hot
    (fp8, a pure position encoding of row_e & 31) into edge-slot order, in
    batch-contiguous DRAM layout; the device STREAMS both with plain
    contiguous DMA on separate engine queues (no gather ucode at all).
  - coef = sw*sigmoid(rep_row+rep_col)*ns_col for all slots is computed once
    up front from 4 streamed bf16 metadata planes; per chunk the one-hot is
    scaled by coef (one DVE op) and used as the matmul lhsT against the raw
    streamed x rows: psum[32q+j, :] += sum_e coef_e*onehot[e,j]*x[col_e].
  - deg is shipped from host (edge bincount clamped to >=1 -- layout
    metadata); 1/deg, sigmoid(rep), and sigmoid(rep)*x_self are computed
    once up front. Finalize per quad group: one ACT copy (agg/deg -> bf16),
    two PE transposes build [agg/deg | srep*x_self]^T directly in PSUM, one
    matmul with [W; W_self] applies both weights, leaky_relu on ACT, DMA out.
    Output rows are re-assembled on host per the block permutation.
"""
import sys

if "/opt/trn_rl_repo" not in sys.path:
    sys.path.insert(0, "/opt/trn_rl_repo")

import numpy as np

P = 128
D = 64
B32 = 32                              # dest block rows
N_NODES = 100000
N_CORES = 8
N_BLK32 = N_NODES // B32              # 3125 global 32-row blocks (exact)
N_SLOT = 392                          # block slots per core (392*8=3136)
N_BLK_T = N_SLOT * N_CORES
N_GRP = N_SLOT // 4                   # 98 psum quad groups per core
BATCH = 64                            # chunks per compute batch
USE_FP8_ONEHOT = True
USE_ACT_LRELU = True


def _build_program(cap):
    """Emit + compile the single-core SPMD program. cap: [N_SLOT] run
    capacities in edges, each a multiple of 128; sum(cap) is a multiple of
    128*BATCH."""
    import concourse.bacc as bacc
    import concourse.mybir as mybir
    import concourse.tile as tile
    from concourse.masks import make_identity

    f32 = mybir.dt.float32
    bf16 = mybir.dt.bfloat16
    oh_dt = mybir.dt.float8e4 if USE_FP8_ONEHOT else bf16

    cap = [int(v) for v in cap]
    C = sum(cap) // P
    assert C % BATCH == 0
    NB = C // BATCH
    chunk_slot = []
    run_first = []
    run_last = []
    pos = 0
    for j, cp in enumerate(cap):
        nch = cp // P
        run_first.append(pos)
        run_last.append(pos + nch - 1)
        chunk_slot.extend([j] * nch)
        pos += nch
    assert pos == C

    nc = bacc.Bacc("TRN2", target_bir_lowering=False, debug=False)

    xexp_d = nc.dram_tensor("xexp", [NB * P, BATCH * D], bf16,
                            kind="ExternalInput")
    oh_d = nc.dram_tensor("oh", [NB * P, BATCH * B32], oh_dt,
                          kind="ExternalInput")
    meta_d = nc.dram_tensor("meta", [P, 4 * C], bf16, kind="ExternalInput")
    deg_d = nc.dram_tensor("deg", [P, N_GRP], f32, kind="ExternalInput")
    repsh_d = nc.dram_tensor("rep_sh", [P, N_GRP], f32, kind="ExternalInput")
    xself_d = nc.dram_tensor("x_self", [P, N_GRP * D], bf16,
                             kind="ExternalInput")
    wcat_d = nc.dram_tensor("w_cat", [2 * D, D], bf16, kind="ExternalInput")
    out_d = nc.dram_tensor("out", [N_GRP * P, D], f32, kind="ExternalOutput")

    AL = mybir.AluOpType
    ACT = mybir.ActivationFunctionType

    with tile.TileContext(nc) as tc:
        with (
            tc.tile_pool(name="meta", bufs=1) as meta,
            tc.tile_pool(name="gather", bufs=3) as gpool,
            tc.tile_pool(name="onehot", bufs=3) as opool,
            tc.tile_pool(name="const", bufs=1) as cpool,
            tc.tile_pool(name="fin", bufs=4) as fpool,
            tc.tile_pool(name="psum", bufs=4, space="PSUM") as psum,
            tc.tile_pool(name="psumT", bufs=2, space="PSUM") as psumT,
        ):
            meta_s = meta.tile([P, 4, C], bf16)
            deg_s = meta.tile([P, N_GRP], f32)
            repsh_s = meta.tile([P, N_GRP], f32)
            xself_s = meta.tile([P, N_GRP, D], bf16)
            coef16 = meta.tile([P, C], bf16)
            recip_all = meta.tile([P, N_GRP], f32)
            srep_all = meta.tile([P, N_GRP], f32)
            xselfS = meta.tile([P, N_GRP, D], bf16)
            wcat_s = cpool.tile([2 * D, D], bf16)
            ident = cpool.tile([P, P], bf16)
            meta_v = meta_d[:].rearrange("p (f c) -> p f c", f=4)
            nc.gpsimd.dma_start(out=deg_s[:], in_=deg_d[:])
            nc.gpsimd.dma_start(out=repsh_s[:], in_=repsh_d[:])
            nc.gpsimd.dma_start(out=xself_s[:].rearrange("p j d -> p (j d)"),
                                in_=xself_d[:])
            nc.gpsimd.dma_start(out=wcat_s[:], in_=wcat_d[:])
            make_identity(nc, ident[:])

            # one-time prep: coef (quartered so batch 0 starts early),
            # 1/deg, sigmoid(rep), srep*x_self
            coef = meta.tile([P, C], f32)
            CQ = C // 4
            for s in range(0, C, CQ):
                e = min(C, s + CQ)
                nc.scalar.dma_start(out=meta_s[:, :, s:e],
                                    in_=meta_v[:, :, s:e])
                nc.vector.tensor_tensor(out=coef[:, s:e],
                                        in0=meta_s[:, 1, s:e],
                                        in1=meta_s[:, 2, s:e], op=AL.add)
                nc.scalar.activation(coef[:, s:e], coef[:, s:e], ACT.Sigmoid)
                nc.vector.tensor_tensor(out=coef[:, s:e], in0=coef[:, s:e],
                                        in1=meta_s[:, 0, s:e], op=AL.mult)
                nc.vector.tensor_tensor(out=coef16[:, s:e], in0=coef[:, s:e],
                                        in1=meta_s[:, 3, s:e], op=AL.mult)
            nc.vector.reciprocal(out=recip_all[:], in_=deg_s[:])
            nc.scalar.activation(srep_all[:], repsh_s[:], ACT.Sigmoid)
            nc.vector.tensor_tensor(
                out=xselfS[:], in0=xself_s[:],
                in1=srep_all[:].rearrange("p (j o) -> p j o", o=1)
                    .to_broadcast([P, N_GRP, D]),
                op=AL.mult)

            run_ps = {}  # group -> live psum tile

            def finalize_group(t):
                ps = run_ps.pop(t)
                cat0 = fpool.tile([P, D], bf16, tag="cat0")
                nc.scalar.activation(cat0[:], ps[:], ACT.Copy,
                                     scale=recip_all[:, t:t + 1])
                catT_ps = psumT.tile([P, P], bf16, tag="catT")
                nc.tensor.transpose(out=catT_ps[0:D, :], in_=cat0[:],
                                    identity=ident[:])
                nc.tensor.transpose(out=catT_ps[D:2 * D, :],
                                    in_=xselfS[:, t, :], identity=ident[:],
                                    tile_position=(0, D))
                catT = fpool.tile([P, P], bf16, tag="catT_s")
                nc.vector.tensor_copy(out=catT[:], in_=catT_ps[:])
                out_ps = psumT.tile([P, D], f32, tag="out_ps")
                nc.tensor.matmul(out=out_ps[:], lhsT=catT[:], rhs=wcat_s[:],
                                 start=True, stop=True)
                outb = fpool.tile([P, D], f32, tag="outb")
                if USE_ACT_LRELU:
                    nc.scalar.activation(outb[:], out_ps[:], ACT.Lrelu,
                                         alpha=0.01)
                else:
                    lk = fpool.tile([P, D], f32, tag="lk")
                    nc.any.tensor_scalar_mul(out=lk[:], in0=out_ps[:],
                                             scalar1=0.01)
                    nc.any.tensor_tensor(out=outb[:], in0=out_ps[:],
                                         in1=lk[:], op=AL.max)
                nc.gpsimd.dma_start(out=out_d[t * P:(t + 1) * P, :],
                                    in_=outb[:])

            for b in range(NB):
                c0 = b * BATCH
                xg = gpool.tile([P, BATCH * D], bf16, tag="xg")
                nc.sync.dma_start(out=xg[:],
                                  in_=xexp_d[b * P:(b + 1) * P, :])
                xg_v = xg[:].rearrange("p (b d) -> p b d", d=D)

                oh8 = opool.tile([P, BATCH * B32], oh_dt, tag="oh8")
                nc.sync.dma_start(out=oh8[:],
                                  in_=oh_d[b * P:(b + 1) * P, :])
                oh8_v = oh8[:].rearrange("p (b n) -> p b n", n=B32)

                ohw = opool.tile([P, BATCH, B32], bf16, tag="ohw")
                nc.vector.tensor_tensor(
                    out=ohw[:], in0=oh8_v,
                    in1=coef16[:, c0:c0 + BATCH]
                        .rearrange("p (b o) -> p b o", o=1)
                        .to_broadcast([P, BATCH, B32]),
                    op=AL.mult)

                for i in range(BATCH):
                    ci = c0 + i
                    j = chunk_slot[ci]
                    t, q = j >> 2, j & 3
                    is_start = ci == run_first[j]
                    is_stop = ci == run_last[j]
                    if is_start and q == 0:
                        run_ps[t] = psum.tile([P, D], f32, tag="agg",
                                              name="agg_ps")
                    nc.tensor.matmul(
                        out=run_ps[t][q * B32:(q + 1) * B32, :],
                        lhsT=ohw[:, i, :], rhs=xg_v[:, i, :],
                        start=is_start, stop=is_stop,
                        tile_position=(0, q * B32))
                    if is_stop and q == 3:
                        finalize_group(t)

    nc.compile()
    return nc


def _preprocess(x, edge_index, sim_weight, rep, node_signal):
    """Host-side layout: deal destination 32-row blocks to cores (snake by
    count), group edges into (core, slot) runs padded to 128-edge chunks,
    pre-expand x[col] (bf16) and the destination one-hot (fp8) into slot
    order, produce per-core arrays + deg/rep/x_self in psum-partition
    layout."""
    import ml_dtypes

    bf16 = ml_dtypes.bfloat16
    oh_np = ml_dtypes.float8_e4m3fn if USE_FP8_ONEHOT else bf16
    row = np.ascontiguousarray(edge_index[0]).astype(np.int64)
    col = np.ascontiguousarray(edge_index[1]).astype(np.int64)
    sw = np.ascontiguousarray(sim_weight).astype(np.float32)
    rep_f = np.ascontiguousarray(rep).astype(np.float32)
    ns_f = np.ascontiguousarray(node_signal).astype(np.float32)
    x16 = np.ascontiguousarray(x).astype(bf16)
    E = row.shape[0]

    gb = row >> 5
    off = (row & 31).astype(np.int64)

    counts = np.bincount(gb, minlength=N_BLK_T).astype(np.int64)
    order_desc = np.argsort(-counts, kind="stable")
    assign = np.empty((N_CORES, N_SLOT), dtype=np.int64)
    for j in range(N_SLOT):
        ids = order_desc[j * N_CORES:(j + 1) * N_CORES]
        if j % 2 == 0:
            assign[:, j] = ids
        else:
            assign[::-1, j] = ids
    inv_core = np.empty(N_BLK_T, dtype=np.int64)
    inv_slot = np.empty(N_BLK_T, dtype=np.int64)
    for c in range(N_CORES):
        inv_core[assign[c]] = c
        inv_slot[assign[c]] = np.arange(N_SLOT)

    cap = ((counts[assign].max(axis=0) + P - 1) // P) * P
    cap = np.maximum(cap, P)
    # pad total capacity to a whole number of BATCH-chunk batches
    capsum = int(cap.sum())
    pad = (-capsum) % (P * BATCH)
    cap[N_SLOT - 1] += pad
    slot_base = np.zeros(N_SLOT + 1, dtype=np.int64)
    np.cumsum(cap, out=slot_base[1:])
    tot_pc = int(slot_base[-1])
    C = tot_pc // P

    core_e = inv_core[gb]
    slot_e = inv_slot[gb]
    key = core_e * N_SLOT + slot_e
    order = np.argsort(key, kind="stable")
    gcounts = np.bincount(key, minlength=N_CORES * N_SLOT)
    gstart = np.zeros(N_CORES * N_SLOT + 1, dtype=np.int64)
    np.cumsum(gcounts, out=gstart[1:])
    rank = np.arange(E, dtype=np.int64) - gstart[key[order]]
    abs_slot = core_e[order] * tot_pc + slot_base[slot_e[order]] + rank

    tot = N_CORES * tot_pc
    xexp = np.zeros((tot, D), dtype=bf16)
    xexp[abs_slot] = x16[col[order]]
    ohx = np.zeros((tot, B32), dtype=oh_np)
    ohx[abs_slot, off[order]] = oh_np(1.0)
    sw_p = np.zeros(tot, dtype=np.float32)
    rr_p = np.zeros(tot, dtype=np.float32)
    rc_p = np.zeros(tot, dtype=np.float32)
    ns_p = np.zeros(tot, dtype=np.float32)
    sw_p[abs_slot] = sw[order]
    rr_p[abs_slot] = rep_f[row[order]]
    rc_p[abs_slot] = rep_f[col[order]]
    ns_p[abs_slot] = ns_f[col[order]]

    NB = C // BATCH
    xexp_t = np.ascontiguousarray(
        xexp.reshape(N_CORES, NB, BATCH, P, D).transpose(0, 1, 3, 2, 4)
        .reshape(N_CORES, NB * P, BATCH * D))
    oh_t = np.ascontiguousarray(
        ohx.reshape(N_CORES, NB, BATCH, P, B32).transpose(0, 1, 3, 2, 4)
        .reshape(N_CORES, NB * P, BATCH * B32))

    def per_core(a):
        return a.reshape(N_CORES, C, P).transpose(0, 2, 1)

    meta_t = np.ascontiguousarray(
        np.stack([per_core(sw_p), per_core(rr_p), per_core(rc_p),
                  per_core(ns_p)], axis=2).reshape(N_CORES, P, 4 * C)
        ).astype(bf16)

    # psum-partition layout grids: row_id(c, p, t) for partition p, group t
    pj = np.arange(P) // B32
    po = np.arange(P) % B32
    slot_grid = (np.arange(N_GRP)[None, :] * 4 + pj[:, None])    # [P, N_GRP]
    gb_grid = assign[:, slot_grid]                               # [8, P, N_GRP]
    rid = gb_grid * B32 + po[None, :, None]                      # [8, P, N_GRP]
    valid = gb_grid < N_BLK32
    rid_c = np.minimum(rid, N_NODES - 1)

    degc = np.maximum(np.bincount(row, minlength=N_NODES), 1).astype(
        np.float32)
    deg_t = np.ascontiguousarray(np.where(valid, degc[rid_c], 1.0))
    repsh_t = np.ascontiguousarray(np.where(valid, rep_f[rid_c], 0.0))
    xself_t = np.ascontiguousarray(
        np.where(valid[..., None], x16[rid_c], bf16(0))
        .reshape(N_CORES, P, N_GRP * D))

    return (cap, rid, valid, xexp_t, oh_t, meta_t, deg_t, repsh_t, xself_t)


_compiled = {}


def _get_program(cap):
    key = tuple(cap.tolist())
    if key not in _compiled:
        _compiled[key] = _build_program(cap)
    return _compiled[key]


def run(x, edge_index, sim_weight, rep, node_signal, W, W_self, trace=False):
    from concourse.bass_utils import run_bass_kernel_spmd
    import ml_dtypes

    (cap, rid, valid, xexp_t, oh_t, meta_t, deg_t, repsh_t,
     xself_t) = _preprocess(x, edge_index, sim_weight, rep, node_signal)
    w_cat = np.ascontiguousarray(
        np.concatenate([np.asarray(W, dtype=np.float32),
                        np.asarray(W_self, dtype=np.float32)],
                       axis=0)).astype(ml_dtypes.bfloat16)
    nc = _get_program(cap)
    in_maps = []
    for c in range(N_CORES):
        in_maps.append({
            "xexp": xexp_t[c],
            "oh": oh_t[c],
            "meta": meta_t[c],
            "deg": deg_t[c],
            "rep_sh": repsh_t[c],
            "x_self": xself_t[c],
            "w_cat": w_cat,
        })
    res = run_bass_kernel_spmd(nc, in_maps, core_ids=list(range(N_CORES)),
                               trace=trace)
    out = np.empty((N_NODES, D), dtype=np.float32)
    for c in range(N_CORES):
        oc = res.results[c]["out"]                 # [N_GRP*P, D]
        ocv = oc.reshape(N_GRP, P, D).transpose(1, 0, 2)  # [P, N_GRP, D]
        out[rid[c][valid[c]]] = ocv[valid[c]]
    return out, res


def kernel(x, edge_index, sim_weight, rep, node_signal, W, W_self):
    out, _ = run(x, edge_index, sim_weight, rep, node_signal, W, W_self)
    return out
